# revision 7
# baseline (speedup 1.0000x reference)
"""CRF loss (nn_EntityModel_crf) Bass/Tile kernel for Trainium2, 8 NeuronCores.

Data-parallel over batch (8 examples/core).  Per core:

feat phase:  hidden is host-pretransposed to hidT [H, BS] fp8e4m3 so
  feat^T = w4^T @ hidT needs no PE transposes (w4 = W tiled 4x on the output
  dim -> psum holds 4 stacked copies of feat^T per example).

scan phase (log-partition): rank-1 chunk factorization.  Each example's 512
  emission factors are split into C=32 chunks of L=16 "pairs" P_s = E^T D_s
  (E = exp(transitions), D_s = diag(exp(feat_s - SHIFT))).  A product of 16
  positive matrices is numerically rank-1 (Birkhoff contraction), so
    Z ~ prod_c (w_c . v_{c-1}) / prod_c (1^T v_c),
  with v_c = N_c 1 (forward chain) and w_c = N_c^T 1 (backward chain), both
  16-round VECTOR recurrences state <- M (f o state), run for all (example,
  chunk, direction) chains simultaneously: 124 columns x 128 partitions
  (rows 0-63 forward for the two examples of a pair, 64-127 backward), one
  [128,~62] bf16 matmul with blockdiag(E,E,E^T,E^T) plus one elementwise
  multiply per round.  Host-measured rel-err of this approximation +
  fp8 emission + bf16 scan vs the fp64 reference: ~2e-5.

gold path: one-hot(tags) multiply-accumulate against the feat psum, with the
  transition term trans[tag_{s-1}, t] pre-accumulated into the same psum by
  one extra matmul per example (start=False).

kernel(**inputs) takes FULL inputs, shards/preps on host, runs via
run_bass_kernel_spmd on cores 0-7, sums per-core partial losses on host.
"""

import numpy as np
import ml_dtypes

import concourse.bass as bass
import concourse.tile as tile
from concourse import mybir
from concourse._compat import with_exitstack
from concourse.bass_utils import run_bass_kernel_spmd

B, S, H, T = 64, 512, 768, 32
NCORES = 8
BL = B // NCORES          # 8 examples per core
BS = BL * S               # 4096
C = 32                    # chunks per example
L = 16                    # steps (rounds) per chunk
W_ = C - 1                # chain columns per pair = 31
SHIFT = 4.125
CORR = SHIFT * S          # 2112, added back to ln(Z) per example

F32 = mybir.dt.float32
F32R = mybir.dt.float32r
BF16 = mybir.dt.bfloat16
FP8 = mybir.dt.float8e4
AF = mybir.ActivationFunctionType
ALU = mybir.AluOpType


@with_exitstack
def _crf_kernel(ctx, tc, out_ap, ins):
    nc = tc.nc

    consts = ctx.enter_context(tc.tile_pool(name="consts", bufs=1))
    persist = ctx.enter_context(tc.tile_pool(name="persist", bufs=1))

    def cload(name, shape, dt=F32):
        t = consts.tile(list(shape), dt, tag=name)
        nc.sync.dma_start(t[:], ins[name])
        return t

    w4sb = consts.tile([128, 6 * 128], FP8, tag="w4")
    nc.sync.dma_start(
        w4sb[:].rearrange("p (k m) -> p k m", k=6),
        ins["w4"].rearrange("(k p) m -> p k m", p=128),
    )
    tagsf_t = cload("tagsf", (128, 1024), BF16)
    transtk_t = cload("transtk", (128, T))          # [E;E;E^T;E^T] log-domain
    trans4_t = cload("trans4", (128, T), BF16)      # trans tiled 4x (gold q)
    iota4_t = cload("iota4", (128, 1))
    bm_t = cload("bm", (128, 1))                    # b_out (4x) - SHIFT
    ones1_t = cload("ones1", (128, 1))
    blk2_t = cload("blk2", (64, 2), BF16)
    pmat_t = cload("pmat", (128, 128), BF16)
    recipE_t = cload("recipE", (128, 1))

    eft = persist.tile([128, 4 * 512], BF16, tag="eft")   # exp(feat-SHIFT) tables
    E4 = persist.tile([128, 128], BF16, tag="E4")         # blockdiag exp(trans)
    ohf = persist.tile([128, 1024], BF16, tag="ohf")      # one-hot, folded layout
    gacc = persist.tile([128, BL], F32, tag="gacc")       # per-ex gold partials
    sfin = persist.tile([128, 4 * W_], BF16, tag="sfin")  # final chain states
    outv = persist.tile([2, 12], F32, tag="outv")
    dummy = persist.tile([1, 2], F32, tag="dummy")

    # ACT Exp table warmup on a low-wait dummy (walrus attaches the
    # ACT_TABLE_LOAD to the first activation of a new table set).
    nc.vector.memset(dummy[:], 0.0)
    nc.scalar.activation(dummy[:, 0:1], dummy[:, 1:2], AF.Exp)

    # E4 = blockdiag(E, E, E^T, E^T); bounce trans through DVE so the ACT
    # exps wait on a single semaphore.
    trc = persist.tile([128, T], F32, tag="trc")
    nc.gpsimd.memset(E4[:], 0.0)
    nc.vector.tensor_copy(trc[:], transtk_t[:])
    for q in range(4):
        sl = slice(32 * q, 32 * q + 32)
        nc.scalar.activation(E4[sl, sl], trc[sl, :], AF.Exp)

    nc.gpsimd.memset(gacc[:], 0.0)
    # one-hot: ohf[32g+t, m] = (tag[1024g+m] == t)
    nc.vector.tensor_scalar(
        ohf[:, 0:512], tagsf_t[:, 0:512], iota4_t[:], None, op0=ALU.is_equal
    )
    nc.gpsimd.tensor_scalar(
        ohf[:, 512:1024], tagsf_t[:, 512:1024], iota4_t[:], None, op0=ALU.is_equal
    )
    # backward-table round-0 pad columns = 1.0 (cols 16w of bwd rows)
    nc.gpsimd.memset(
        eft[64:128, :].rearrange("p (pp w l) -> p pp w l", pp=4, l=L)[:, :, :, 0:1],
        1.0,
    )

    scan_a_work = []  # deferred emission closures for scan group A

    def emit_feat(ex, hidp, psf, scrp):
        g, h = ex // 2, ex % 2
        hid_t = hidp.tile([128, 6 * 512], FP8, tag="hid")
        nc.sync.dma_start(
            hid_t[:].rearrange("p (k s) -> p k s", k=6),
            ins["hidT"].rearrange("(k p) s -> p k s", p=128)[
                :, :, 512 * ex : 512 * (ex + 1)
            ],
        )
        hv = hid_t[:].rearrange("p (k s) -> p k s", k=6)
        ps = psf.tile([128, 512], F32, tag="psf")
        for k in range(6):
            nc.tensor.matmul(
                ps[:],
                w4sb[:, 128 * k : 128 * (k + 1)],
                hv[:, k],
                start=(k == 0),
                stop=(k == 5),
            )
        fr = slice(32 * h, 32 * h + 32)          # forward table rows
        br = slice(64 + 32 * h, 96 + 32 * h)     # backward table rows
        pc = 512 * g                             # pair column offset in eft
        # forward table: natural s order, cols 0..496 (497: col 496 is the
        # final-mult factor f_{16*31} for the last backward chunk)
        nc.scalar.activation(
            eft[fr, pc : pc + 497], ps[fr, 0:497], AF.Exp, bias=bm_t[fr, :]
        )
        # backward table: block w cols 1..15 = f[16w+32-j] (reversed)
        dst = eft[br, pc : pc + 512].rearrange("p (w l) -> p w l", l=L)[:, 0:31, 1:16]
        src = ps[br, :].rearrange("p (w l) -> p w l", l=L)[:, 1:32, 15:0:-1]
        nc.scalar.activation(dst, src, AF.Exp, bias=bm_t[br, :])
        # gold: accumulate trans[tag_{s-1}, t] into ps rows 32g, cols 1..511
        qr = slice(32 * g, 32 * g + 32)
        nc.tensor.matmul(
            ps[qr, 1:512],
            trans4_t[qr, :],
            ohf[qr, 512 * h : 512 * h + 511],
            start=False,
            stop=True,
            skip_group_check=True,
            tile_position=(32 * g, 32 * g),
        )
        scr = scrp.tile([32, 512], BF16, tag="scr")
        nc.vector.scalar_tensor_tensor(
            scr[:],
            ohf[qr, 512 * h : 512 * h + 512],
            0.0,
            ps[qr, 0:512],
            op0=ALU.add,
            op1=ALU.mult,
            accum_out=gacc[qr, ex : ex + 1],
        )
        if h == 1:
            # f'_511 = f_511 / rowsum(E): fix-up col 481 of this pair's bwd rows
            col = pc + 16 * 30 + 1
            nc.vector.tensor_scalar(
                eft[64:128, col : col + 1],
                eft[64:128, col : col + 1],
                recipE_t[64:128, :],
                None,
                op0=ALU.mult,
            )

    def eft_ap(grp, j):
        return eft[:, 1024 * grp : 1024 * (grp + 1)].rearrange(
            "p (pp w l) -> p pp w l", pp=2, l=L
        )[:, :, 0:31, j : j + 1]

    def emit_scan_round(grp, j, pss, scp, state):
        if j == 0:
            ps = pss.tile([128, 2 * W_], F32, tag=f"pss{grp}")
            nc.tensor.matmul(ps[:], E4[:], eft_ap(grp, 0), start=True, stop=True)
            return ps
        sc = scp.tile([128, 2 * W_], BF16, tag=f"sc{grp}")
        scv = sc[:].rearrange("p (pp w) -> p pp w", pp=2).unsqueeze(3)
        if grp == 0:
            # DVE reads PSUM directly
            stv = state[:].rearrange("p (pp w) -> p pp w", pp=2).unsqueeze(3)
            nc.vector.tensor_tensor(scv, stv, eft_ap(grp, j), op=ALU.mult)
        else:
            # gpsimd cannot access PSUM: bounce via ACT copy, multiply on Pool
            cp = scp.tile([128, 2 * W_], BF16, tag="cpB")
            nc.scalar.copy(cp[:], state[:])
            cpv = cp[:].rearrange("p (pp w) -> p pp w", pp=2).unsqueeze(3)
            nc.gpsimd.tensor_tensor(scv, cpv, eft_ap(grp, j), op=ALU.mult)
        ps = pss.tile([128, 2 * W_], F32, tag=f"pss{grp}")
        nc.tensor.matmul(ps[:], E4[:], sc[:], start=True, stop=True)
        return ps

    def emit_combine(grp, state, cps):
        half = slice(62 * grp, 62 * (grp + 1))
        if grp == 0:
            nc.vector.tensor_copy(sfin[:, half], state[:])
        else:
            nc.scalar.copy(sfin[:, half], state[:])
        psW = cps.tile([128, 2 * W_], F32, tag="c")
        nc.tensor.matmul(psW[:], pmat_t[:], sfin[:, half], start=True, stop=True)
        # dots = v_{c-1} o (w_c permuted) o f_fin ; f_fin = fwd col 16(w+1)
        ffap = eft[0:64, 1024 * grp : 1024 * (grp + 1)].rearrange(
            "p (pp w l) -> p pp w l", pp=2, l=L
        )[:, :, 1:32, 0:1]
        tmp = persist.tile([64, 2 * W_], BF16, tag=f"tmp{grp}")
        tv = tmp[:].rearrange("p (pp w) -> p pp w", pp=2).unsqueeze(3)
        nc.vector.tensor_tensor(tv, psW[0:64, :].rearrange("p (pp w) -> p pp w", pp=2).unsqueeze(3), ffap, op=ALU.mult)
        dots = persist.tile([64, 2 * W_], BF16, tag=f"dots{grp}")
        nc.gpsimd.tensor_tensor(dots[:], tmp[:], sfin[0:64, half], op=ALU.mult)
        psDt = cps.tile([128, 2 * W_], F32, tag="c")
        psD = psDt[0:2, :]
        nc.tensor.matmul(psD[:], blk2_t[:], dots[:], start=True, stop=True)
        psAt = cps.tile([128, 2 * W_], F32, tag="c")
        psA = psAt[0:2, :]
        nc.tensor.matmul(psA[:], blk2_t[:], sfin[0:64, half], start=True, stop=True)
        if grp == 0:
            # Ln table-set load lands on this low-wait dummy
            nc.scalar.activation(dummy[:, 1:2], psD[0:1, 0:1], AF.Ln)
        lnd = persist.tile([2, 2 * W_], F32, tag=f"lnd{grp}")
        nc.scalar.activation(
            lnd[:], psD[:], AF.Ln, accum_out=outv[:, 2 * grp : 2 * grp + 1]
        )
        lna = persist.tile([2, 60], F32, tag=f"lna{grp}")
        aap = psA[:].rearrange("p (pp w) -> p pp w", pp=2)[:, :, 1:31]
        nc.scalar.activation(
            lna[:].rearrange("p (pp w) -> p pp w", pp=2),
            aap,
            AF.Ln,
            accum_out=outv[:, 2 * grp + 1 : 2 * grp + 2],
        )

    with (
        tc.tile_pool(name="hidp", bufs=3) as hidp,
        tc.tile_pool(name="scrp", bufs=2) as scrp,
        tc.tile_pool(name="scpA", bufs=2) as scpA,
        tc.tile_pool(name="scpB", bufs=2) as scpB,
        tc.tile_pool(name="psf", bufs=2, space="PSUM") as psf,
        tc.tile_pool(name="pssA", bufs=2, space="PSUM") as pssA,
        tc.tile_pool(name="pssB", bufs=2, space="PSUM") as pssB,
        tc.tile_pool(name="cps", bufs=2, space="PSUM") as cps,
    ):
        for ex in range(4):
            emit_feat(ex, hidp, psf, scrp)
        stA = None
        for ex in range(4, 8):
            emit_feat(ex, hidp, psf, scrp)
            for j in range(4 * (ex - 4), 4 * (ex - 3)):
                stA = emit_scan_round(0, j, pssA, scpA, stA)
        emit_combine(0, stA, cps)
        stB = None
        for j in range(L):
            stB = emit_scan_round(1, j, pssB, scpB, stB)
        emit_combine(1, stB, cps)

        psGt = cps.tile([128, 2 * W_], F32, tag="c")
        psG = psGt[0:1, 0:BL]
        nc.tensor.matmul(psG[:], ones1_t[:], gacc[:], start=True, stop=True)
        nc.vector.tensor_copy(outv[0:1, 4:12], psG[:])
        nc.sync.dma_start(out_ap, outv[:])


# walrus codegen accepts only one sync-wait per compute instruction; hoist
# extras onto same-engine NoOps (CoreSim path skips this).
_MULTIWAIT_OK = {"InstAllEngineBarrier", "InstEventSemaphore"}


def _split_sync_waits(nc):
    nid = [0]
    for fn in nc.m.functions:
        for blk in fn.blocks:
            out = []
            changed = False
            for inst in blk.instructions:
                si = inst.sync_info
                waits = list(si.on_wait) if si and si.on_wait else []
                if len(waits) > 1 and type(inst).__name__ not in _MULTIWAIT_OK:
                    changed = True
                    for w in waits[:-1]:
                        nop = mybir.InstNoOp(name=f"I-wsplit-{nid[0]}")
                        nid[0] += 1
                        nop.engine = inst.engine
                        nop.sync_info = mybir.SyncInfo(on_wait=[w], on_update=[])
                        out.append(nop)
                    inst.sync_info = mybir.SyncInfo(
                        on_wait=[waits[-1]], on_update=list(si.on_update or [])
                    )
                out.append(inst)
            if changed:
                try:
                    blk.instructions = out
                except Exception:
                    del blk.instructions[:]
                    blk.instructions.extend(out)


_NC_CACHE = []


def build_module(for_hw=True, repeat=1):
    nc = bass.Bass(
        "TRN2", target_bir_lowering=False, debug=False, num_devices=NCORES
    )
    shapes = {
        "hidT": ((H, BS), FP8),
        "w4": ((H, 128), FP8),
        "tagsf": ((128, 1024), BF16),
        "transtk": ((128, T), F32),
        "trans4": ((128, T), BF16),
        "iota4": ((128, 1), F32),
        "bm": ((128, 1), F32),
        "ones1": ((128, 1), F32),
        "blk2": ((64, 2), BF16),
        "pmat": ((128, 128), BF16),
        "recipE": ((128, 1), F32),
    }
    ins = {
        name: nc.dram_tensor(name, list(shape), dt, kind="ExternalInput").ap()
        for name, (shape, dt) in shapes.items()
    }
    out = nc.dram_tensor("outv", [2, 12], F32, kind="ExternalOutput").ap()
    with tile.TileContext(nc) as tc:
        if repeat > 1:
            with tc.For_i(0, repeat, 1):
                _crf_kernel(tc, out, ins)
        else:
            _crf_kernel(tc, out, ins)
    if for_hw:
        _split_sync_waits(nc)
    return nc


def make_in_maps(hidden, mask, target_tag, W_out, b_out, transitions):
    f8 = ml_dtypes.float8_e4m3fn
    bf = ml_dtypes.bfloat16
    hidden = np.asarray(hidden, dtype=np.float32)
    mask = np.asarray(mask)
    tags = np.where(mask != 0, target_tag, T).astype(np.float32)      # [B, S]
    trans = np.asarray(transitions, np.float32)
    E = np.exp(trans)
    recipE = np.ones((128, 1), np.float32)
    recipE[64:128, 0] = np.tile(1.0 / E.sum(axis=1), 2)
    blk2 = np.zeros((64, 2), np.float32)
    blk2[0:32, 0] = 1.0
    blk2[32:64, 1] = 1.0
    pmat = np.zeros((128, 128), np.float32)
    pmat[64:128, 0:64] = np.eye(64)
    shared = {
        "w4": np.ascontiguousarray(np.tile(np.asarray(W_out, np.float32), (1, 4))).astype(f8),
        "transtk": np.ascontiguousarray(
            np.concatenate([np.tile(trans, (2, 1)), np.tile(trans.T, (2, 1))], axis=0)
        ),
        "trans4": np.ascontiguousarray(np.tile(trans, (4, 1))).astype(bf),
        "iota4": np.ascontiguousarray(np.tile(np.arange(T, dtype=np.float32), 4)[:, None]),
        "bm": (np.tile(np.asarray(b_out, np.float32), 4)[:, None] - SHIFT),
        "ones1": np.ones((128, 1), np.float32),
        "blk2": blk2.astype(bf),
        "pmat": pmat.astype(bf),
        "recipE": recipE,
    }
    in_maps = []
    for c in range(NCORES):
        hid = hidden[c * BL : (c + 1) * BL].reshape(BS, H)
        m = dict(shared)
        m["hidT"] = np.ascontiguousarray(hid.T).astype(f8)
        tg = tags[c * BL : (c + 1) * BL].reshape(4, 1024)
        m["tagsf"] = np.ascontiguousarray(np.repeat(tg, 32, axis=0)).astype(bf)
        in_maps.append(m)
    return in_maps


def kernel(hidden, mask, target_tag, W_out, b_out, transitions):
    if not _NC_CACHE:
        _NC_CACHE.append(build_module())
    nc = _NC_CACHE[0]
    in_maps = make_in_maps(hidden, mask, target_tag, W_out, b_out, transitions)
    res = run_bass_kernel_spmd(nc, in_maps, core_ids=list(range(NCORES)))
    total = 0.0
    for r in res.results:
        o = np.asarray(r["outv"], dtype=np.float64)
        total += o[0, 0] + o[1, 0] + o[0, 2] + o[1, 2]      # sum ln dots
        total -= o[0, 1] + o[1, 1] + o[0, 3] + o[1, 3]      # sum ln alphas
        total += BL * CORR
        total -= o[0, 4:12].sum()                            # gold scores
    return np.float32(total)
